# revision 12
# baseline (speedup 1.0000x reference)
"""Trainium2 Bass kernel for factorized space-time attention.

Computation (per batch b of 8, one NeuronCore each):
  qkv = x @ w_qkv.T                      (3136, 2304)
  heads 0-5:  spatial attention over 196 patches within each of 16 frames
  heads 6-11: temporal attention over groups of 16 consecutive tokens
              (raw-reshape semantics of the reference: groups of 16
               consecutive n within each (b, head) slice)
  out = concat(head outputs) @ w_proj.T + b_proj

Strategy: data-parallel over batch (8 cores). All activations kept
feature-major ([d, n]) on chip so every matmul contraction runs over the
partition dim with no on-device transposes; x / weights are pre-transposed
host-side. V is additionally produced in token-major (natural) layout
directly by flipping the projection matmul orientation, chunked two ways:
per-frame [128+68] rows for spatial heads, uniform 112-row windows
(= 7 temporal groups) for temporal heads. Temporal attention is computed
on 112x112 score windows with a block-diagonal mask (7 x 16x16).
Softmax skips the max-subtraction (scores are ~N(0,1); exp is safe in
fp32) and normalizes via a ones-matmul partition-broadcast of 1/rowsum.
"""

import sys

# concourse normally comes from the axon site tree (sitecustomize); the
# append is a fallback so a bare environment still finds it.
if "/opt/trn_rl_repo" not in sys.path:
    sys.path.append("/opt/trn_rl_repo")

import numpy as np

import concourse.bass as bass  # noqa: F401  (engine namespaces live on nc)
import concourse.mybir as mybir
import concourse.tile as tile
from concourse import bacc
from concourse.bass_utils import run_bass_kernel_spmd

F32 = mybir.dt.float32
BF16 = mybir.dt.bfloat16
AF = mybir.ActivationFunctionType

# problem dims (hardcoded per contract)
B = 8
F = 16
P = 196
D = 768
NH = 12
HD = 64
N = F * P  # 3136
E3 = 3 * D  # 2304
SB = 784  # superblock = lcm(196, 16) tokens
NSB = N // SB  # 4
FPSB = SB // P  # 4 frames per superblock
WPSB = SB // 112  # 7 temporal windows per superblock
SCALE = HD ** -0.5

# compute dtype for matmul inputs ("f32" safest, "bf16" 2x faster on PE)
COMPUTE = "f32"

_CACHE = {}


def _build(compute: str):
    """Build + bass-compile the per-core kernel. Returns the Bacc object."""
    cdt = F32 if compute == "f32" else BF16
    nc = bacc.Bacc("TRN2", target_bir_lowering=False, debug=False, num_devices=B)

    xt_d = nc.dram_tensor("xt", (D, N), cdt, kind="ExternalInput")
    wqkv_d = nc.dram_tensor("wqkvT", (D, E3), cdt, kind="ExternalInput")
    wproj_d = nc.dram_tensor("wprojT", (D, D), cdt, kind="ExternalInput")
    bias_d = nc.dram_tensor("bias", (D, 1), F32, kind="ExternalInput")
    mask_d = nc.dram_tensor("mask", (112, 112), cdt, kind="ExternalInput")
    out_d = nc.dram_tensor("outT", (D, N), F32, kind="ExternalOutput")

    with tile.TileContext(nc) as tc:
        with (
            tc.tile_pool(name="const", bufs=1) as cpool,
            tc.tile_pool(name="work", bufs=1) as wpool,
            tc.tile_pool(name="small", bufs=4) as spool,
            tc.tile_pool(name="psum", bufs=2, space="PSUM") as ppool,
        ):
            # ---- constants -------------------------------------------------
            wq = []
            for dc in range(6):
                t = cpool.tile([128, E3], cdt, tag=f"wq{dc}", name=f"wq{dc}")
                nc.sync.dma_start(t[:], wqkv_d.ap()[128 * dc : 128 * (dc + 1), :])
                wq.append(t)
            wp = []
            for dc in range(6):
                t = cpool.tile([128, D], cdt, tag=f"wp{dc}", name=f"wp{dc}")
                nc.sync.dma_start(t[:], wproj_d.ap()[128 * dc : 128 * (dc + 1), :])
                wp.append(t)
            bias_t = cpool.tile([128, 6], F32, tag="bias", name="bias_t")
            nc.sync.dma_start(
                bias_t[:], bias_d.ap().rearrange("(e p) one -> p (e one)", p=128)
            )
            mask_t = cpool.tile([112, 112], cdt, tag="mask", name="mask_t")
            nc.sync.dma_start(mask_t[:], mask_d.ap())
            ones_row = cpool.tile([1, 128], cdt, tag="ones_r", name="ones_row")
            nc.gpsimd.memset(ones_row[:], 1.0)
            ones_col = cpool.tile([128, 1], cdt, tag="ones_c", name="ones_col")
            nc.gpsimd.memset(ones_col[:], 1.0)
            zeros_col = cpool.tile([128, 1], F32, tag="zeros_c", name="zeros_col")
            nc.gpsimd.memset(zeros_col[:], 0.0)

            for s in range(NSB):
                so = SB * s  # superblock token offset

                # ---- load x^T superblock ----------------------------------
                xts = []
                for dc in range(6):
                    t = wpool.tile([128, SB], cdt, tag=f"xts{dc}", name=f"xts{dc}_{s}")
                    nc.sync.dma_start(
                        t[:], xt_d.ap()[128 * dc : 128 * (dc + 1), so : so + SB]
                    )
                    xts.append(t)

                # ---- QKV projection: Q,K regions, feature-major -----------
                # qkvt[t] rows = features 128t..128t+127 of [Q(768) | K(768)]
                qkvt = []
                for ti in range(12):
                    qt = wpool.tile([128, SB], cdt, tag=f"qkvt{ti}", name=f"qkvt{ti}_{s}")
                    for j in range(2):
                        ps = ppool.tile([128, 392], F32, tag="mm", name=f"ps_qk{s}_{ti}_{j}")
                        for dc in range(6):
                            nc.tensor.matmul(
                                ps[:],
                                wq[dc][:, 128 * ti : 128 * (ti + 1)],
                                xts[dc][:, 392 * j : 392 * (j + 1)],
                                start=(dc == 0),
                                stop=(dc == 5),
                            )
                        nc.scalar.copy(qt[:, 392 * j : 392 * (j + 1)], ps[:])
                    qkvt.append(qt)

                # ---- V projection, token-major (natural) ------------------
                # spatial V: per-frame chunks of [128, 68] rows; cols = heads 0-5
                vs = []
                for f in range(FPSB):
                    for ci, (m0, msz) in enumerate(((0, 128), (128, 68))):
                        vt_ = wpool.tile(
                            [msz, 384], cdt, tag=f"vs{f}_{ci}", name=f"vs{f}_{ci}_{s}"
                        )
                        ps = ppool.tile([msz, 384], F32, tag="mm", name=f"ps_vs{s}_{f}_{ci}")
                        for dc in range(6):
                            nc.tensor.matmul(
                                ps[:],
                                xts[dc][:, 196 * f + m0 : 196 * f + m0 + msz],
                                wq[dc][:, 1536:1920],
                                start=(dc == 0),
                                stop=(dc == 5),
                            )
                        nc.scalar.copy(vt_[:], ps[:])
                        vs.append(vt_)
                # temporal V: uniform 112-token windows; cols = heads 6-11
                vt = []
                for w in range(WPSB):
                    vt_ = wpool.tile([112, 384], cdt, tag=f"vt{w}", name=f"vt{w}_{s}")
                    ps = ppool.tile([112, 384], F32, tag="mm", name=f"ps_vt{s}_{w}")
                    for dc in range(6):
                        nc.tensor.matmul(
                            ps[:],
                            xts[dc][:, 112 * w : 112 * (w + 1)],
                            wq[dc][:, 1920:2304],
                            start=(dc == 0),
                            stop=(dc == 5),
                        )
                    nc.scalar.copy(vt_[:], ps[:])
                    vt.append(vt_)

                # ---- attention output, feature-major ----------------------
                attnT = [
                    wpool.tile([128, SB], cdt, tag=f"attnT{i}", name=f"attnT{i}_{s}")
                    for i in range(6)
                ]

                # ---- spatial attention (heads 0-5, per frame) --------------
                for f in range(FPSB):
                    fo = 196 * f
                    for hp in range(3):
                        for hi in range(2):
                            h = 2 * hp + hi
                            pb = 64 * hi
                            ps_av = ppool.tile(
                                [64, 196], F32, tag="av", name=f"ps_sav{s}_{f}_{h}"
                            )
                            qtile = qkvt[h // 2]
                            ktile = qkvt[6 + h // 2]
                            # scores^T and exp, per contraction chunk
                            es = []
                            for ci, (m0, msz) in enumerate(((0, 128), (128, 68))):
                                ps_st = ppool.tile(
                                    [msz, 196], F32, tag="st",
                                    name=f"ps_st{s}_{f}_{h}_{ci}",
                                )
                                nc.tensor.matmul(
                                    ps_st[:],
                                    ktile[pb : pb + 64, fo + m0 : fo + m0 + msz],
                                    qtile[pb : pb + 64, fo : fo + 196],
                                    start=True,
                                    stop=True,
                                )
                                e = spool.tile(
                                    [msz, 196], cdt, tag="e", name=f"e{s}_{f}_{h}_{ci}"
                                )
                                nc.scalar.activation(
                                    e[:], ps_st[:], AF.Exp,
                                    bias=zeros_col[:msz, :], scale=SCALE,
                                )
                                es.append(e)
                            # O^T numerator
                            for ci, (m0, msz) in enumerate(((0, 128), (128, 68))):
                                nc.tensor.matmul(
                                    ps_av[:],
                                    vs[2 * f + ci][:, 64 * h : 64 * (h + 1)],
                                    es[ci][:],
                                    start=(ci == 0),
                                    stop=(ci == 1),
                                )
                            # column sums of exp scores
                            ps_sum = ppool.tile(
                                [1, 196], F32, tag="sum", name=f"ps_ssum{s}_{f}_{h}"
                            )
                            for ci, (m0, msz) in enumerate(((0, 128), (128, 68))):
                                nc.tensor.matmul(
                                    ps_sum[:],
                                    ones_col[:msz, :],
                                    es[ci][:],
                                    start=(ci == 0),
                                    stop=(ci == 1),
                                )
                            r = spool.tile([1, 196], cdt, tag="r", name=f"r{s}_{f}_{h}")
                            nc.vector.reciprocal(r[:], ps_sum[:])
                            ps_b = ppool.tile(
                                [64, 196], F32, tag="st", name=f"ps_sb{s}_{f}_{h}"
                            )
                            nc.tensor.matmul(
                                ps_b[:], ones_row[:, :64], r[:], start=True, stop=True
                            )
                            rb = spool.tile(
                                [64, 196], F32, tag="rb", name=f"rb{s}_{f}_{h}"
                            )
                            nc.scalar.copy(rb[:], ps_b[:])
                            # normalize; even heads write attnT rows 0-63
                            # directly, odd heads go via a tmp tile + an
                            # SBUF->SBUF DMA (only DMA can shift partitions)
                            if hi == 0:
                                nc.vector.tensor_mul(
                                    attnT[h // 2][0:64, fo : fo + 196],
                                    ps_av[:], rb[:],
                                )
                            else:
                                tmp = spool.tile(
                                    [64, 196], F32, tag="tmp", name=f"tm{s}_{f}_{h}"
                                )
                                nc.vector.tensor_mul(tmp[:], ps_av[:], rb[:])
                                nc.sync.dma_start(
                                    attnT[h // 2][64:128, fo : fo + 196], tmp[:]
                                )

                # ---- temporal attention (heads 6-11, per 112-window) -------
                for w in range(WPSB):
                    wo = 112 * w
                    for hp in range(3):
                        for hi in range(2):
                            h = 6 + 2 * hp + hi  # global head 6..11
                            pb = 64 * hi
                            ps_av = ppool.tile(
                                [64, 112], F32, tag="av", name=f"ps_tav{s}_{w}_{h}"
                            )
                            qtile = qkvt[h // 2]
                            ktile = qkvt[6 + h // 2]
                            ps_st = ppool.tile(
                                [112, 112], F32, tag="st", name=f"ps_tst{s}_{w}_{h}"
                            )
                            nc.tensor.matmul(
                                ps_st[:],
                                ktile[pb : pb + 64, wo : wo + 112],
                                qtile[pb : pb + 64, wo : wo + 112],
                                start=True,
                                stop=True,
                            )
                            e = spool.tile(
                                [112, 112], cdt, tag="e", name=f"et{s}_{w}_{h}"
                            )
                            nc.scalar.activation(
                                e[:], ps_st[:], AF.Exp,
                                bias=zeros_col[:112, :], scale=SCALE,
                            )
                            em = spool.tile(
                                [112, 112], cdt, tag="e", name=f"em{s}_{w}_{h}"
                            )
                            nc.vector.tensor_mul(em[:], e[:], mask_t[:])
                            nc.tensor.matmul(
                                ps_av[:],
                                vt[w][:, 64 * (h - 6) : 64 * (h - 5)],
                                em[:],
                                start=True,
                                stop=True,
                            )
                            ps_sum = ppool.tile(
                                [1, 112], F32, tag="sum", name=f"ps_tsum{s}_{w}_{h}"
                            )
                            nc.tensor.matmul(
                                ps_sum[:], ones_col[:112, :], em[:],
                                start=True, stop=True,
                            )
                            r = spool.tile([1, 112], cdt, tag="r", name=f"rt{s}_{w}_{h}")
                            nc.vector.reciprocal(r[:], ps_sum[:])
                            ps_b = ppool.tile(
                                [64, 112], F32, tag="st", name=f"ps_tb{s}_{w}_{h}"
                            )
                            nc.tensor.matmul(
                                ps_b[:], ones_row[:, :64], r[:], start=True, stop=True
                            )
                            rb = spool.tile(
                                [64, 112], F32, tag="rb", name=f"rbt{s}_{w}_{h}"
                            )
                            nc.scalar.copy(rb[:], ps_b[:])
                            at = attnT[3 + (h - 6) // 2]
                            if hi == 0:
                                nc.vector.tensor_mul(
                                    at[0:64, wo : wo + 112], ps_av[:], rb[:]
                                )
                            else:
                                tmp = spool.tile(
                                    [64, 112], F32, tag="tmp", name=f"tmt{s}_{w}_{h}"
                                )
                                nc.vector.tensor_mul(tmp[:], ps_av[:], rb[:])
                                nc.sync.dma_start(
                                    at[64:128, wo : wo + 112], tmp[:]
                                )

                # ---- output projection + bias ------------------------------
                for ec in range(6):
                    for j in range(2):
                        ps = ppool.tile([128, 392], F32, tag="mm", name=f"ps_o{s}_{ec}_{j}")
                        for dc in range(6):
                            nc.tensor.matmul(
                                ps[:],
                                wp[dc][:, 128 * ec : 128 * (ec + 1)],
                                attnT[dc][:, 392 * j : 392 * (j + 1)],
                                start=(dc == 0),
                                stop=(dc == 5),
                            )
                        ot = spool.tile([128, 392], F32, tag="ot", name=f"ot{s}_{ec}_{j}")
                        nc.scalar.activation(
                            ot[:], ps[:], AF.Identity,
                            bias=bias_t[:, ec : ec + 1], scale=1.0,
                        )
                        nc.sync.dma_start(
                            out_d.ap()[
                                128 * ec : 128 * (ec + 1),
                                so + 392 * j : so + 392 * (j + 1),
                            ],
                            ot[:],
                        )

    nc.compile()
    return nc


def _get_nc(compute: str):
    if compute not in _CACHE:
        _CACHE[compute] = _build(compute)
    return _CACHE[compute]


def _np_dtype(compute: str):
    if compute == "f32":
        return np.float32
    import ml_dtypes

    return ml_dtypes.bfloat16


def kernel(x, w_qkv, w_proj, b_proj):
    nc = _get_nc(COMPUTE)
    dt = _np_dtype(COMPUTE)

    x = np.asarray(x, dtype=np.float32).reshape(B, N, D)
    xT = np.ascontiguousarray(x.transpose(0, 2, 1)).astype(dt)  # (B, D, N)
    wqkvT = np.ascontiguousarray(np.asarray(w_qkv, np.float32).T).astype(dt)
    wprojT = np.ascontiguousarray(np.asarray(w_proj, np.float32).T).astype(dt)
    bias = np.asarray(b_proj, np.float32).reshape(D, 1)

    mask = np.zeros((112, 112), np.float32)
    for g in range(7):
        mask[16 * g : 16 * (g + 1), 16 * g : 16 * (g + 1)] = 1.0
    mask = mask.astype(dt)

    in_maps = [
        {"xt": xT[b], "wqkvT": wqkvT, "wprojT": wprojT, "bias": bias, "mask": mask}
        for b in range(B)
    ]
    res = run_bass_kernel_spmd(nc, in_maps, core_ids=list(range(B)))
    out = np.stack([r["outT"].T for r in res.results])  # (B, N, D)
    return np.ascontiguousarray(out.reshape(B, F, P, D)).astype(np.float32)


if __name__ == "__main__":
    rng = np.random.default_rng(0)
    x = rng.standard_normal((B, F, P, D), dtype=np.float32)
    w_qkv = rng.standard_normal((E3, D), dtype=np.float32) * D**-0.5
    w_proj = rng.standard_normal((D, D), dtype=np.float32) * D**-0.5
    b_proj = np.zeros(D, np.float32)
    out = kernel(x=x, w_qkv=w_qkv, w_proj=w_proj, b_proj=b_proj)
    print(out.shape, out.dtype)


# revision 37
# speedup vs baseline: 2875.4444x; 2875.4444x over previous
"""Trainium2 Bass kernel for factorized space-time attention.

Computation (per batch b of 8, one NeuronCore each):
  qkv = x @ w_qkv.T                      (3136, 2304)
  heads 0-5:  spatial attention over 196 patches within each of 16 frames
  heads 6-11: temporal attention over groups of 16 consecutive tokens
              (raw-reshape semantics of the reference: groups of 16
               consecutive n within each (b, head) slice)
  out = concat(head outputs) @ w_proj.T + b_proj

Strategy: data-parallel over batch (8 cores). All activations kept
feature-major ([d, n]) on chip so every matmul contraction runs over the
partition dim with no on-device transposes; x / weights are pre-transposed
host-side. V is additionally produced in token-major (natural) layout
directly by flipping the projection matmul orientation, chunked two ways:
per-frame [128+68] rows for spatial heads, uniform 112-row windows
(= 7 temporal groups) for temporal heads. Temporal attention is computed
on 112x112 score windows with a block-diagonal mask (7 x 16x16).
Softmax skips the max-subtraction (scores are ~N(0,1); exp is safe in
fp32) and normalizes via a ones-matmul partition-broadcast of 1/rowsum.
"""

import sys

# concourse normally comes from the axon site tree (sitecustomize); the
# append is a fallback so a bare environment still finds it.
if "/opt/trn_rl_repo" not in sys.path:
    sys.path.append("/opt/trn_rl_repo")

import numpy as np

import concourse.bass as bass  # noqa: F401  (engine namespaces live on nc)
import concourse.mybir as mybir
import concourse.tile as tile
from concourse import bacc
from concourse.bass_utils import run_bass_kernel_spmd

F32 = mybir.dt.float32
BF16 = mybir.dt.bfloat16
AF = mybir.ActivationFunctionType

# problem dims (hardcoded per contract)
B = 8
F = 16
P = 196
D = 768
NH = 12
HD = 64
N = F * P  # 3136
E3 = 3 * D  # 2304
SB = 784  # superblock = lcm(196, 16) tokens
NSB = N // SB  # 4
FPSB = SB // P  # 4 frames per superblock
WPSB = SB // 112  # 7 temporal windows per superblock
SCALE = HD ** -0.5

# compute dtype for matmul inputs ("f32" safest, "bf16" 2x faster on PE)
COMPUTE = "f32"

_CACHE = {}


def _build(compute: str, reps: int = 1):
    """Build + bass-compile the per-core kernel. Returns the Bacc object.

    compute: "f32" | "f32r" | "bf16" — dtype of matmul inputs. "f32r" keeps
    all data fp32 but runs the three projection matmul groups in the PE's
    faster reduced-precision fp32 mode via operand bitcasts.
    reps: device-side repetition count (for timing; wraps the body in For_i).
    """
    cdt = BF16 if compute == "bf16" else F32
    F32R = mybir.dt.float32r

    def mmcast(ap):
        return ap.bitcast(F32R) if compute == "f32r" else ap

    # bf16 tiles are half-size; spend the freed SBUF on double-buffering the
    # big per-superblock tiles so consecutive superblocks overlap fully.
    wb = 2 if compute == "bf16" else 1

    nc = bacc.Bacc("TRN2", target_bir_lowering=False, debug=False, num_devices=B)

    xt_d = nc.dram_tensor("xt", (D, N), cdt, kind="ExternalInput")
    wqkv_d = nc.dram_tensor("wqkvT", (D, E3), cdt, kind="ExternalInput")
    wproj_d = nc.dram_tensor("wprojT", (D, D), cdt, kind="ExternalInput")
    bias_d = nc.dram_tensor("bias", (D, 1), F32, kind="ExternalInput")
    mask_d = nc.dram_tensor("mask", (112, 112), cdt, kind="ExternalInput")
    out_d = nc.dram_tensor("outT", (D, N), F32, kind="ExternalOutput")

    with tile.TileContext(nc) as tc:
        with (
            tc.tile_pool(name="const", bufs=1) as cpool,
            tc.tile_pool(name="work", bufs=1) as wpool,
            tc.tile_pool(name="small", bufs=4) as spool,
            tc.tile_pool(name="psum", bufs=2, space="PSUM") as ppool,
        ):
            # ---- constants -------------------------------------------------
            wq = []
            for dc in range(6):
                t = cpool.tile([128, E3], cdt, tag=f"wq{dc}", name=f"wq{dc}")
                nc.sync.dma_start(t[:], wqkv_d.ap()[128 * dc : 128 * (dc + 1), :])
                wq.append(t)
            wp = []
            for dc in range(6):
                t = cpool.tile([128, D], cdt, tag=f"wp{dc}", name=f"wp{dc}")
                nc.sync.dma_start(t[:], wproj_d.ap()[128 * dc : 128 * (dc + 1), :])
                wp.append(t)
            bias_t = cpool.tile([128, 6], F32, tag="bias", name="bias_t")
            nc.sync.dma_start(
                bias_t[:], bias_d.ap().rearrange("(e p) one -> p (e one)", p=128)
            )
            mask2_t = cpool.tile([112, 224], cdt, tag="mask", name="mask2_t")
            nc.sync.dma_start(mask2_t[:, 0:112], mask_d.ap())
            nc.sync.dma_start(mask2_t[:, 112:224], mask_d.ap())
            zeros_col = cpool.tile([128, 1], F32, tag="zeros_c", name="zeros_col")
            nc.gpsimd.memset(zeros_col[:], 0.0)
            # row 64 of ones (matching the psum row the softmax sums land on)
            # is the stationary operand of the 1/sum partition-broadcast matmul
            ones64 = cpool.tile([65, 64], F32, tag="ones64", name="ones64")
            nc.gpsimd.memset(ones64[:], 1.0)

            import contextlib

            rep_ctx = tc.For_i(0, reps, 1) if reps > 1 else contextlib.nullcontext()
            with rep_ctx:
              for s in range(NSB):
                so = SB * s  # superblock token offset

                # ---- load x^T superblock ----------------------------------
                xts = []
                for dc in range(6):
                    t = wpool.tile([128, SB], cdt, tag=f"xts{dc}", bufs=wb, name=f"xts{dc}_{s}")
                    nc.sync.dma_start(
                        t[:], xt_d.ap()[128 * dc : 128 * (dc + 1), so : so + SB]
                    )
                    xts.append(t)

                # ---- QKV projection: Q,K regions, feature-major -----------
                # qkvt[t] rows = features 128t..128t+127 of [Q(768) | K(768)]
                qkvt = []
                for ti in range(12):
                    qt = wpool.tile([128, SB], cdt, tag=f"qkvt{ti}", bufs=wb, name=f"qkvt{ti}_{s}")
                    for j in range(2):
                        ps = ppool.tile([128, 392], F32, tag="mm", bufs=2, name=f"ps_qk{s}_{ti}_{j}")
                        for dc in range(6):
                            nc.tensor.matmul(
                                ps[:],
                                mmcast(wq[dc][:, 128 * ti : 128 * (ti + 1)]),
                                mmcast(xts[dc][:, 392 * j : 392 * (j + 1)]),
                                start=(dc == 0),
                                stop=(dc == 5),
                            )
                        nc.scalar.copy(qt[:, 392 * j : 392 * (j + 1)], ps[:])
                    qkvt.append(qt)

                # ---- V projection, token-major (natural) ------------------
                # layout per tile: 6 heads x [64 V-cols | ones-col] = 390 cols;
                # the ones column makes the AV matmul (M=65) emit the softmax
                # denominator as output row 64 for free.
                def v_proj(msz, tok0, wcol0, vtag, vname, psname):
                    vt_ = wpool.tile([msz, 390], cdt, tag=vtag, bufs=wb, name=vname)
                    ps = ppool.tile([msz, 384], F32, tag="mm", bufs=2, name=psname)
                    for dc in range(6):
                        nc.tensor.matmul(
                            ps[:],
                            mmcast(xts[dc][:, tok0 : tok0 + msz]),
                            mmcast(wq[dc][:, wcol0 : wcol0 + 384]),
                            start=(dc == 0),
                            stop=(dc == 5),
                        )
                    nc.scalar.copy(
                        vt_.rearrange("p (h c) -> p h c", c=65)[:, :, 0:64],
                        ps.rearrange("p (h c) -> p h c", c=64),
                    )
                    nc.gpsimd.memset(
                        vt_.rearrange("p (h c) -> p h c", c=65)[:, :, 64:65], 1.0
                    )
                    return vt_

                # spatial V: per-frame chunks of [128, 68] rows; cols = heads 0-5
                vs = []
                for f in range(FPSB):
                    for ci, (m0, msz) in enumerate(((0, 128), (128, 68))):
                        vs.append(
                            v_proj(msz, 196 * f + m0, 1536, f"vs{f}_{ci}",
                                   f"vs{f}_{ci}_{s}", f"ps_vs{s}_{f}_{ci}")
                        )
                # temporal V: uniform 112-token windows; cols = heads 6-11
                vt = []
                for w in range(WPSB):
                    vt.append(
                        v_proj(112, 112 * w, 1920, f"vt{w}",
                               f"vt{w}_{s}", f"ps_vt{s}_{w}")
                    )

                # ---- attention output, feature-major ----------------------
                attnT = [
                    wpool.tile([128, SB], cdt, tag=f"attnT{i}", bufs=wb,
                               name=f"attnT{i}_{s}")
                    for i in range(6)
                ]

                # ---- spatial attention (heads 0-5, per frame) --------------
                # one psum tile per accumulation group (HW requires a start/
                # stop group to own its bank); pairs share the 1/sum
                # reciprocal+broadcast stage.
                for f in range(FPSB):
                    fo = 196 * f
                    for hp in range(3):
                        ps_avs = []
                        for hi in range(2):
                            h = 2 * hp + hi
                            pb = 64 * hi
                            qtile = qkvt[h // 2]
                            ktile = qkvt[6 + h // 2]
                            es = []
                            for ci, (m0, msz) in enumerate(((0, 128), (128, 68))):
                                ps_st = ppool.tile(
                                    [msz, 196], F32, tag="st", bufs=3,
                                    name=f"ps_st{s}_{f}_{h}_{ci}",
                                )
                                nc.tensor.matmul(
                                    ps_st[:],
                                    ktile[pb : pb + 64, fo + m0 : fo + m0 + msz],
                                    qtile[pb : pb + 64, fo : fo + 196],
                                    start=True,
                                    stop=True,
                                )
                                e = spool.tile(
                                    [msz, 196], cdt, tag="e", bufs=6,
                                    name=f"e{s}_{f}_{h}_{ci}",
                                )
                                nc.scalar.activation(
                                    e[:], ps_st[:], AF.Exp,
                                    bias=zeros_col[:msz, :], scale=SCALE,
                                )
                                es.append(e)
                            # O^T numerator rows 0-63, softmax denom row 64
                            ps_av = ppool.tile(
                                [65, 196], F32, tag="av", bufs=2,
                                name=f"ps_sav{s}_{f}_{h}",
                            )
                            for ci in range(2):
                                nc.tensor.matmul(
                                    ps_av[:],
                                    vs[2 * f + ci][:, 65 * h : 65 * h + 65],
                                    es[ci][:],
                                    start=(ci == 0),
                                    stop=(ci == 1),
                                )
                            ps_avs.append(ps_av)
                        r = spool.tile([65, 392], F32, tag="r", name=f"r{s}_{f}_{hp}")
                        for hi in range(2):
                            nc.vector.reciprocal(
                                r[64:65, 196 * hi : 196 * hi + 196],
                                ps_avs[hi][64:65, :],
                            )
                        ps_b = ppool.tile(
                            [64, 392], F32, tag="mm", bufs=2, name=f"ps_b{s}_{f}_{hp}"
                        )
                        nc.tensor.matmul(
                            ps_b[:], ones64[64:65, :], r[64:65, :],
                            start=True, stop=True,
                        )
                        rb = spool.tile([64, 392], F32, tag="rb", name=f"rb{s}_{f}_{hp}")
                        nc.scalar.copy(rb[:], ps_b[:])
                        for hi in range(2):
                            h = 2 * hp + hi
                            cs = slice(196 * hi, 196 * hi + 196)
                            # even heads write attnT rows 0-63 directly, odd
                            # heads via tmp + SBUF->SBUF DMA (partition shift)
                            if hi == 0:
                                nc.vector.tensor_mul(
                                    attnT[h // 2][0:64, fo : fo + 196],
                                    ps_avs[hi][0:64, :], rb[:, cs],
                                )
                            else:
                                tmp = spool.tile(
                                    [64, 196], cdt, tag="tmp", name=f"tm{s}_{f}_{h}"
                                )
                                nc.vector.tensor_mul(
                                    tmp[:], ps_avs[hi][0:64, :], rb[:, cs]
                                )
                                nc.sync.dma_start(
                                    attnT[h // 2][64:128, fo : fo + 196], tmp[:]
                                )

                # ---- temporal attention (heads 6-11, per 112-window) -------
                for w in range(WPSB):
                    wo = 112 * w
                    for hp in range(3):
                        ps_avs = []
                        for hi in range(2):
                            h = 6 + 2 * hp + hi  # global head 6..11
                            pb = 64 * hi
                            ps_st = ppool.tile(
                                [112, 112], F32, tag="st", bufs=3,
                                name=f"ps_tst{s}_{w}_{h}",
                            )
                            nc.tensor.matmul(
                                ps_st[:],
                                qkvt[6 + h // 2][pb : pb + 64, wo : wo + 112],
                                qkvt[h // 2][pb : pb + 64, wo : wo + 112],
                                start=True,
                                stop=True,
                            )
                            e = spool.tile(
                                [112, 112], cdt, tag="e", bufs=6,
                                name=f"et{s}_{w}_{h}",
                            )
                            nc.scalar.activation(
                                e[:], ps_st[:], AF.Exp,
                                bias=zeros_col[:112, :], scale=SCALE,
                            )
                            em = spool.tile(
                                [112, 112], cdt, tag="e", bufs=6,
                                name=f"em{s}_{w}_{h}",
                            )
                            nc.vector.tensor_mul(em[:], e[:], mask2_t[:, 0:112])
                            ps_av = ppool.tile(
                                [65, 112], F32, tag="av", bufs=2,
                                name=f"ps_tav{s}_{w}_{h}",
                            )
                            nc.tensor.matmul(
                                ps_av[:],
                                vt[w][:, 65 * (h - 6) : 65 * (h - 6) + 65],
                                em[:],
                                start=True,
                                stop=True,
                            )
                            ps_avs.append(ps_av)
                        r = spool.tile([65, 224], F32, tag="r", name=f"rt{s}_{w}_{hp}")
                        for hi in range(2):
                            nc.vector.reciprocal(
                                r[64:65, 112 * hi : 112 * hi + 112],
                                ps_avs[hi][64:65, :],
                            )
                        ps_b = ppool.tile(
                            [64, 224], F32, tag="mm", bufs=2, name=f"ps_tb{s}_{w}_{hp}"
                        )
                        nc.tensor.matmul(
                            ps_b[:], ones64[64:65, :], r[64:65, :],
                            start=True, stop=True,
                        )
                        rb = spool.tile([64, 224], F32, tag="rb", name=f"rbt{s}_{w}_{hp}")
                        nc.scalar.copy(rb[:], ps_b[:])
                        for hi in range(2):
                            h = 6 + 2 * hp + hi
                            cs = slice(112 * hi, 112 * hi + 112)
                            at = attnT[3 + (h - 6) // 2]
                            if hi == 0:
                                nc.vector.tensor_mul(
                                    at[0:64, wo : wo + 112], ps_avs[hi][0:64, :],
                                    rb[:, cs],
                                )
                            else:
                                tmp = spool.tile(
                                    [64, 112], cdt, tag="tmp", name=f"tmt{s}_{w}_{h}"
                                )
                                nc.vector.tensor_mul(
                                    tmp[:], ps_avs[hi][0:64, :], rb[:, cs]
                                )
                                nc.sync.dma_start(
                                    at[64:128, wo : wo + 112], tmp[:]
                                )

                # ---- output projection + bias ------------------------------
                for ec in range(6):
                    for j in range(2):
                        ps = ppool.tile([128, 392], F32, tag="mm", bufs=2, name=f"ps_o{s}_{ec}_{j}")
                        for dc in range(6):
                            nc.tensor.matmul(
                                ps[:],
                                mmcast(wp[dc][:, 128 * ec : 128 * (ec + 1)]),
                                mmcast(attnT[dc][:, 392 * j : 392 * (j + 1)]),
                                start=(dc == 0),
                                stop=(dc == 5),
                            )
                        ot = spool.tile([128, 392], F32, tag="ot", name=f"ot{s}_{ec}_{j}")
                        nc.scalar.activation(
                            ot[:], ps[:], AF.Identity,
                            bias=bias_t[:, ec : ec + 1], scale=1.0,
                        )
                        nc.sync.dma_start(
                            out_d.ap()[
                                128 * ec : 128 * (ec + 1),
                                so + 392 * j : so + 392 * (j + 1),
                            ],
                            ot[:],
                        )

    nc.compile()
    return nc


def _get_nc(compute: str):
    if compute not in _CACHE:
        _CACHE[compute] = _build(compute)
    return _CACHE[compute]


def _np_dtype(compute: str):
    if compute == "f32":
        return np.float32
    import ml_dtypes

    return ml_dtypes.bfloat16


def kernel(x, w_qkv, w_proj, b_proj):
    nc = _get_nc(COMPUTE)
    dt = _np_dtype(COMPUTE)

    x = np.asarray(x, dtype=np.float32).reshape(B, N, D)
    xT = np.ascontiguousarray(x.transpose(0, 2, 1)).astype(dt)  # (B, D, N)
    wqkvT = np.ascontiguousarray(np.asarray(w_qkv, np.float32).T).astype(dt)
    wprojT = np.ascontiguousarray(np.asarray(w_proj, np.float32).T).astype(dt)
    bias = np.asarray(b_proj, np.float32).reshape(D, 1)

    mask = np.zeros((112, 112), np.float32)
    for g in range(7):
        mask[16 * g : 16 * (g + 1), 16 * g : 16 * (g + 1)] = 1.0
    mask = mask.astype(dt)

    in_maps = [
        {"xt": xT[b], "wqkvT": wqkvT, "wprojT": wprojT, "bias": bias, "mask": mask}
        for b in range(B)
    ]
    res = run_bass_kernel_spmd(nc, in_maps, core_ids=list(range(B)))
    out = np.stack([r["outT"].T for r in res.results])  # (B, N, D)
    return np.ascontiguousarray(out.reshape(B, F, P, D)).astype(np.float32)


if __name__ == "__main__":
    rng = np.random.default_rng(0)
    x = rng.standard_normal((B, F, P, D), dtype=np.float32)
    w_qkv = rng.standard_normal((E3, D), dtype=np.float32) * D**-0.5
    w_proj = rng.standard_normal((D, D), dtype=np.float32) * D**-0.5
    b_proj = np.zeros(D, np.float32)
    out = kernel(x=x, w_qkv=w_qkv, w_proj=w_proj, b_proj=b_proj)
    print(out.shape, out.dtype)
